# revision 38
# baseline (speedup 1.0000x reference)
"""Amortized-VI loss kernel for 8 TRN2 NeuronCores (self-contained).

Pure data-parallel over N=524288 samples (65536/core). The scalar loss
decomposes into 3 global sums computed on-chip; host combines 8 per-core
partials:
  S_sq  = sum_{n,p} [ sum_i c_i*xi_i^2 + 5000*((y0-px)^2 + (y1-py)^2) ]
  S_ent = sum_n sum_i ln(softplus(ld_i))
  final = -S_sq/(N*P) + S_ent/N + C

Per core, pipelined in 4 "pairs" (each = 4 MLP super-chunks + 2 particle
chunks) so the per-sample MLP (PE+ACT) overlaps the per-(n,p) DVE work:
  Phase A: 3 tiny MLPs via 2-group block-diag matmuls; relu+bias on ACT;
    outputs bounced through DRAM (fm_dram) with columns ordered
    (g_local, s') so phase-B chunks depend only on their own super-chunks.
  Phase B: einsum xi = mu + L z on DVE with step-0 broadcast APs,
    int-round range reduction + Sin/Abs on ACT, fused square+accumulate
    via scalar_tensor_tensor accum_out.
"""
import sys
import numpy as np
import ml_dtypes

sys.path.insert(0, '/opt/trn_rl_repo')

import concourse.bass as bass
import concourse.tile as tile
from concourse import bacc, mybir
from concourse.bass_utils import run_bass_kernel_spmd
from contextlib import ExitStack

F32 = mybir.dt.float32
F32R = mybir.dt.float32r
BF = mybir.dt.bfloat16
I32 = mybir.dt.int32
AF = mybir.ActivationFunctionType
OP = mybir.AluOpType

NCORES = 8
N_TOT = 524288
NS = N_TOT // NCORES          # 65536 samples/core
P = 8                         # particles
G = 128                       # partition groups (g = n // 512)
S = NS // G                   # 512 samples per group
NCH = 8                       # phase-B chunks (s-slices of 64)
CH = S // NCH                 # 64
PPAIRS = NCH // 2             # 4 pipelined pairs
SC = 16                       # phase-A super-chunks (4 per pair)
SPS = S // SC                 # 32 s-columns per super-chunk
SCW = 64 * SPS                # 2048 matmul cols (64 g_local x 32 s')
TPS = SCW // 512              # 4 matmul tiles per super-chunk

PI = float(np.pi)
TWO_PI = 2.0 * PI
C_PRIOR = [8.0, 2.0, 2.0, 2.0]   # 0.5/sigma_i^2
C_LIK = 5000.0                   # 0.5/noise^2

# L channel order: L00=d0; L10=lo0 L11=d1; L20=lo1 L21=lo2 L22=d2;
# L30=lo3 L31=lo4 L32=lo5 L33=d3
L_TERMS = {0: [(0, 'd0')],
           1: [(0, 'lo0'), (1, 'd1')],
           2: [(0, 'lo1'), (1, 'lo2'), (2, 'd2')],
           3: [(0, 'lo3'), (1, 'lo4'), (2, 'lo5'), (3, 'd3')]}

CHANS = (['mu0', 'mu1', 'mu2', 'mu3'] + [f'ld{i}' for i in range(4)]
         + [f'lo{i}' for i in range(6)])

_CACHE = {}
B3C = [0.0] * 14   # per-channel L3 bias, set by kernel() before build


def _build_nc():
    key = ('nc',) + tuple(B3C)
    if key in _CACHE:
        return _CACHE[key]
    nc = bacc.Bacc("TRN2", target_bir_lowering=False, debug=False)
    dp = lambda n, s: nc.declare_dram_parameter(n, s, F32, isOutput=False)
    dpb = lambda n, s: nc.declare_dram_parameter(n, s, BF, isOutput=False)
    y_fm = dpb("y_fm", [2, NS])
    zs = dp("zs", [NS, 32])
    lhsT1 = dpb("lhsT1", [4, 120])
    lhsT2 = dpb("lhsT2", [120, 60])
    lhsT3 = dpb("lhsT3", [60, 28])
    b1blk = dp("b1blk", [120, 1])
    b2blk = dp("b2blk", [60, 1])
    b3blk = dp("b3blk", [28, 1])
    out_d = nc.declare_dram_parameter("out", [1, 2], F32, isOutput=True)
    fm_dram = nc.dram_tensor("fm_dram", [SC, 28, SCW], BF)

    with tile.TileContext(nc) as tc:
        with ExitStack() as ctx:
            wpool = ctx.enter_context(tc.tile_pool(name="wpool", bufs=1))
            apool = ctx.enter_context(tc.tile_pool(name="apool", bufs=2))
            ps1p = ctx.enter_context(tc.tile_pool(name="ps1", bufs=2, space="PSUM"))
            ps2p = ctx.enter_context(tc.tile_pool(name="ps2", bufs=1, space="PSUM"))
            ps3p = ctx.enter_context(tc.tile_pool(name="ps3", bufs=1, space="PSUM"))
            ctp = ctx.enter_context(tc.tile_pool(name="ctp", bufs=3))
            zpool = ctx.enter_context(tc.tile_pool(name="zpool", bufs=3))
            bpool = ctx.enter_context(tc.tile_pool(name="bpool", bufs=1))
            b2pool = ctx.enter_context(tc.tile_pool(name="b2pool", bufs=2))

            # ---- weights + constants ----
            w1 = wpool.tile([4, 120], BF); nc.sync.dma_start(w1[:], lhsT1[:])
            w2 = wpool.tile([120, 60], BF); nc.sync.dma_start(w2[:], lhsT2[:])
            w3 = wpool.tile([60, 28], BF); nc.sync.dma_start(w3[:], lhsT3[:])
            b1 = wpool.tile([120, 1], F32); nc.sync.dma_start(b1[:], b1blk[:])
            b2 = wpool.tile([60, 1], F32); nc.sync.dma_start(b2[:], b2blk[:])
            kone = wpool.tile([128, 1], F32); nc.vector.memset(kone[:], 1.0)
            khpi = wpool.tile([128, 1], F32); nc.vector.memset(khpi[:], PI / 2)
            accsq = wpool.tile([G, 48], F32); nc.vector.memset(accsq[:], 0.0)
            accent = wpool.tile([G, 16], F32); nc.vector.memset(accent[:], 0.0)

            # y_fm views
            yg = [y_fm[k:k + 1, :].rearrange("a (g s) -> (a g) s", g=G)
                  for k in range(2)]          # [128, 512] DRAM view
            ysc = [y_fm[k:k + 1, :].rearrange(
                "a (grp gl s) -> (a grp) gl s", grp=2, gl=64) for k in range(2)]

            for pp in range(PPAIRS):
                # ===== Phase A: 4 super-chunks =====
                for j in range(TPS):
                    sc = pp * 4 + j
                    yT = apool.tile([4, SCW], BF, tag="yT")
                    for k in range(2):
                        nc.sync.dma_start(
                            yT[2 * k:2 * k + 2, :].rearrange(
                                "r (gl s) -> r gl s", gl=64),
                            ysc[k][:, :, sc * SPS:(sc + 1) * SPS])
                    h1 = apool.tile([120, SCW], BF, tag="h1")
                    h2 = apool.tile([60, SCW], BF, tag="h2")
                    fm = apool.tile([28, SCW], BF, tag="fm")
                    for t2 in range(TPS // 2):
                        cs = slice(t2 * 1024, (t2 + 1) * 1024)
                        p1 = ps1p.tile([120, 1024], F32, tag="p1")
                        for u in range(2):
                            nc.tensor.matmul(p1[:, u * 512:(u + 1) * 512], w1[:],
                                             yT[:, t2 * 1024 + u * 512:
                                                t2 * 1024 + (u + 1) * 512],
                                             start=True, stop=True)
                        nc.scalar.activation(h1[:, cs], p1[:], AF.Relu, bias=b1[:])
                    for t2 in range(TPS // 2):
                        cs = slice(t2 * 1024, (t2 + 1) * 1024)
                        p2 = ps2p.tile([60, 1024], F32, tag="p2")
                        for u in range(2):
                            nc.tensor.matmul(p2[:, u * 512:(u + 1) * 512], w2[:],
                                             h1[:, t2 * 1024 + u * 512:
                                                t2 * 1024 + (u + 1) * 512],
                                             start=True, stop=True)
                        nc.scalar.activation(h2[:, cs], p2[:], AF.Relu, bias=b2[:])
                    for t2 in range(TPS // 2):
                        cs = slice(t2 * 1024, (t2 + 1) * 1024)
                        p3 = ps3p.tile([28, 1024], F32, tag="p3")
                        for u in range(2):
                            nc.tensor.matmul(p3[:, u * 512:(u + 1) * 512], w3[:],
                                             h2[:, t2 * 1024 + u * 512:
                                                t2 * 1024 + (u + 1) * 512],
                                             start=True, stop=True)
                        nc.scalar.activation(fm[:, cs], p3[:], AF.Copy)
                    nc.sync.dma_start(fm_dram[sc], fm[:])

                # ===== gather pair channel tiles (coalesced) =====
                call = ctp.tile([G, 14, 2 * CH], BF, tag="call")
                for j in range(4):
                    sc = 4 * pp + j
                    for grp in range(2):
                        src = fm_dram[sc, 14 * grp:14 * grp + 14, :]\
                            .rearrange("r (gl sp) -> gl r sp", sp=SPS)
                        dst = call[grp * 64:(grp + 1) * 64, :,
                                   j * SPS:(j + 1) * SPS]
                        eng = nc.sync if (j % 2 == 0) else nc.gpsimd
                        eng.dma_start(dst, src)
                cpt = {chan: call[:, ci] for ci, chan in enumerate(CHANS)}
                y0p = ctp.tile([G, 2 * CH], BF, tag="y0p")
                y1p = ctp.tile([G, 2 * CH], BF, tag="y1p")
                nc.gpsimd.dma_start(y0p[:], yg[0][:, pp * 2 * CH:(pp + 1) * 2 * CH])
                nc.gpsimd.dma_start(y1p[:], yg[1][:, pp * 2 * CH:(pp + 1) * 2 * CH])

                # pre-bias mu and lo channels (b3 immediates), bf16
                mub = {}
                lob = {}
                for i in range(4):
                    mt = ctp.tile([G, 2 * CH], BF, tag=f"mub{i}", name=f"mub{i}")
                    nc.vector.tensor_scalar_add(mt[:], cpt[f'mu{i}'], float(B3C[i]))
                    mub[i] = mt
                for i in range(6):
                    lt_ = ctp.tile([G, 2 * CH], BF, tag=f"lob{i}", name=f"lob{i}")
                    nc.vector.tensor_scalar_add(lt_[:], cpt[f'lo{i}'],
                                                float(B3C[8 + i]))
                    lob[i] = lt_

                # ===== softplus(ld+b) -> diag + entropy (per pair) =====
                dgp = {}
                ldb = {}
                ab2 = {}
                for i in range(4):
                    lt = ctp.tile([G, 2 * CH], F32, tag=f"ldb{i}", name=f"ldb{i}")
                    at_ = ctp.tile([G, 2 * CH], F32, tag=f"spab{i}", name=f"spab{i}")
                    nc.vector.tensor_scalar_add(lt[:], cpt[f'ld{i}'],
                                                float(B3C[4 + i]))
                    nc.vector.scalar_tensor_tensor(at_[:], lt[:], -1.0, lt[:],
                                                   OP.mult, OP.max)
                    ldb[i] = lt; ab2[i] = at_
                exn = {}
                lnd = {}
                if True:
                    for i in range(4):
                        et = ctp.tile([G, 2 * CH], F32, tag=f"spex{i}",
                                      name=f"spex{i}")
                        nc.scalar.activation(et[:], ab2[i][:], AF.Exp, scale=-1.0)
                        exn[i] = et
                    ln1 = {}
                    for i in range(4):
                        lt1 = ctp.tile([G, 2 * CH], F32, tag=f"spln{i}",
                                       name=f"spln{i}")
                        nc.scalar.activation(lt1[:], exn[i][:], AF.Ln,
                                             bias=kone[:])
                        ln1[i] = lt1
                    for i in range(4):
                        dgt = ctp.tile([G, 2 * CH], BF, tag=f"dg{i}",
                                       name=f"dg{i}")
                        rel = ctp.tile([G, 2 * CH], F32, tag=f"sp_re{i}",
                                       name=f"sp_re{i}")
                        nc.vector.tensor_scalar_max(rel[:], ldb[i][:], 0.0)
                        nc.vector.tensor_add(dgt[:], rel[:], ln1[i][:])
                        dgp[i] = dgt
                    for i in range(4):
                        lt2 = ctp.tile([G, 2 * CH], F32, tag=f"splnd{i}",
                                       name=f"splnd{i}")
                        nc.scalar.activation(lt2[:], dgp[i][:], AF.Ln)
                        lnd[i] = lt2
                for i in range(4):
                    nc.vector.tensor_reduce(accent[:, pp * 4 + i: pp * 4 + i + 1],
                                            lnd[i][:], mybir.AxisListType.X, OP.add)

                # ===== Phase B: 2 chunks =====
                for cl in range(2):
                    c = pp * 2 + cl
                    col = slice(cl * CH, (cl + 1) * CH)
                    zt = zpool.tile([G, CH, P, 4], BF, tag="zt")
                    nc.gpsimd.dma_start(
                        zt[:], zs[:].rearrange("(g c s) j -> g c (s j)",
                                               g=G, c=NCH)[:, c]
                        .rearrange("g (s p j) -> g s p j", p=P, j=4))

                    def bc(t):   # [G, 2CH] pair tile/AP -> [G, 8, CH] bcast
                        return t[:, col].unsqueeze(1).broadcast_to([G, P, CH])

                    # deinterleave z on GPSIMD: dense bf16 z_j tiles
                    zd = zpool.tile([G, 4, P, CH], BF, tag="zd")
                    for j in range(4):
                        nc.gpsimd.tensor_copy(
                            zd[:, j], zt[:, :, :, j].transpose([0, 2, 1]))

                    def zj(j):   # [G, 8, CH] dense z_j
                        return zd[:, j]

                    def lch(src):
                        t = dgp[int(src[1])] if src[0] == 'd' else lob[int(src[2])]
                        return t[:, col].unsqueeze(1).broadcast_to([G, P, CH])

                    atile = b2pool.tile([G, 3, P, CH], BF, tag="atile")
                    xi0 = b2pool.tile([G, P, CH], BF, tag="xi0")
                    xi2 = b2pool.tile([G, P, CH], BF, tag="xi2")
                    xi3 = b2pool.tile([G, P, CH], BF, tag="xi3")
                    qt = b2pool.tile([G, P, CH], BF, tag="qt")
                    # einsum xi_i = mu_i + sum_j L_ij z_j
                    # i=1 goes straight into atile[:,0]
                    for i, dstap in ((0, xi0[:]), (1, atile[:, 0]),
                                     (2, xi2[:]), (3, xi3[:])):
                        acc = None
                        for ti, (j, src) in enumerate(L_TERMS[i]):
                            tgt = dstap if acc is None else qt[:]
                            eng = nc.gpsimd if (i == 3 and ti < 2) else nc.vector
                            eng.tensor_tensor(tgt, zj(j), lch(src), OP.mult)
                            if acc is not None:
                                nc.vector.tensor_tensor(dstap, dstap, qt[:], OP.add)
                            acc = dstap
                        nc.vector.tensor_tensor(dstap, dstap, bc(mub[i]), OP.add)
                    nc.vector.tensor_tensor(atile[:, 1], atile[:, 0], xi2[:], OP.add)
                    nc.vector.tensor_tensor(atile[:, 2], atile[:, 1], xi3[:], OP.add)
                    # range reduction: k = round(a/2pi), q = a - 2pi*k
                    ki = b2pool.tile([G, 3, P, CH], I32, tag="ki")
                    qt3 = b2pool.tile([G, 3, P, CH], F32, tag="qt3")
                    nc.vector.tensor_scalar(ki[:], atile[:], float(1.0 / TWO_PI),
                                            None, OP.mult)
                    nc.vector.scalar_tensor_tensor(qt3[:], ki[:], -TWO_PI,
                                                   atile[:], OP.mult, OP.add)
                    st = b2pool.tile([G, 3, P, CH], BF, tag="st")
                    ab = b2pool.tile([G, 3, P, CH], F32, tag="ab")
                    co = b2pool.tile([G, 3, P, CH], BF, tag="co")
                    nc.scalar.activation(st[:], qt3[:], AF.Sin)
                    nc.scalar.activation(ab[:], qt3[:], AF.Abs)
                    nc.scalar.activation(co[:], ab[:], AF.Sin, bias=khpi[:],
                                         scale=-1.0)

                    sa = lambda k: st[:, k]
                    ca = lambda k: co[:, k]
                    uu = b2pool.tile([G, P, CH], BF, tag="uu")
                    t1 = b2pool.tile([G, P, CH], BF, tag="t1")
                    ex = b2pool.tile([G, P, CH], BF, tag="ex")
                    sq = b2pool.tile([G, P, CH], BF, tag="sq")
                    nc.vector.tensor_tensor(uu[:], ca(0), ca(1), OP.add)
                    nc.vector.scalar_tensor_tensor(t1[:], ca(2), 2.0, uu[:],
                                                   OP.mult, OP.add)
                    nc.vector.scalar_tensor_tensor(ex[:], t1[:], -0.5, bc(y0p),
                                                   OP.mult, OP.add)
                    nc.vector.scalar_tensor_tensor(
                        sq[:], ex[:], C_LIK, ex[:], OP.mult, OP.mult,
                        accum_out=accsq[:, c * 6: c * 6 + 1])
                    nc.vector.tensor_tensor(uu[:], sa(0), sa(1), OP.add)
                    nc.vector.scalar_tensor_tensor(t1[:], sa(2), 2.0, uu[:],
                                                   OP.mult, OP.add)
                    nc.vector.scalar_tensor_tensor(ex[:], xi0[:], -1.0, bc(y1p),
                                                   OP.mult, OP.add)
                    nc.vector.scalar_tensor_tensor(t1[:], t1[:], -0.5, ex[:],
                                                   OP.mult, OP.add)
                    nc.vector.scalar_tensor_tensor(
                        sq[:], t1[:], C_LIK, t1[:], OP.mult, OP.mult,
                        accum_out=accsq[:, c * 6 + 1: c * 6 + 2])
                    for i, xt in ((0, xi0[:]), (1, atile[:, 0]),
                                  (2, xi2[:]), (3, xi3[:])):
                        nc.vector.scalar_tensor_tensor(
                            sq[:], xt, C_PRIOR[i], xt, OP.mult, OP.mult,
                            accum_out=accsq[:, c * 6 + 2 + i: c * 6 + 3 + i])

            # ===== final reduction =====
            red = wpool.tile([G, 2], F32)
            nc.vector.tensor_reduce(red[:, 0:1], accsq[:], mybir.AxisListType.X,
                                    OP.add)
            nc.vector.tensor_reduce(red[:, 1:2], accent[:], mybir.AxisListType.X,
                                    OP.add)
            pf = ps3p.tile([1, 2], F32, tag="p3")
            nc.tensor.matmul(pf[:], kone[:], red[:], start=True, stop=True)
            ob = wpool.tile([1, 2], F32)
            nc.vector.tensor_copy(ob[:], pf[:])
            nc.sync.dma_start(out_d[:], ob[:])

    nc.compile()
    _CACHE[key] = nc
    return nc


def _pack_host(inp):
    cat = np.concatenate
    W1c = cat([inp['mu_W1'], inp['ld_W1'], inp['lo_W1']], axis=1)      # [2,60]
    b1c = cat([inp['mu_b1'], inp['ld_b1'], inp['lo_b1']])              # [60]
    lhsT1 = np.zeros((4, 120), np.float32)
    for k in range(2):
        for g in range(2):
            lhsT1[2 * k + g, 60 * g:60 * (g + 1)] = W1c[k]
    b1blk = np.tile(b1c, 2).reshape(120, 1).astype(np.float32)

    def blkdiag(ws):
        r = sum(w.shape[0] for w in ws); c = sum(w.shape[1] for w in ws)
        out = np.zeros((r, c), np.float32)
        ro = co = 0
        for w in ws:
            out[ro:ro + w.shape[0], co:co + w.shape[1]] = w
            ro += w.shape[0]; co += w.shape[1]
        return out

    W2b = blkdiag([inp['mu_W2'], inp['ld_W2'], inp['lo_W2']])          # [60,30]
    b2c = cat([inp['mu_b2'], inp['ld_b2'], inp['lo_b2']])              # [30]
    lhsT2 = np.zeros((120, 60), np.float32)
    lhsT2[0:60, 0:30] = W2b; lhsT2[60:120, 30:60] = W2b
    b2blk = np.tile(b2c, 2).reshape(60, 1).astype(np.float32)

    W3b = blkdiag([inp['mu_W3'], inp['ld_W3'], inp['lo_W3']])          # [30,14]
    lhsT3 = np.zeros((60, 28), np.float32)
    lhsT3[0:30, 0:14] = W3b; lhsT3[30:60, 14:28] = W3b
    b3c = cat([inp['mu_b3'], inp['ld_b3'], inp['lo_b3']])              # [14]
    b3blk = np.tile(b3c, 2).reshape(28, 1).astype(np.float32)
    bft = ml_dtypes.bfloat16
    return dict(lhsT1=lhsT1.astype(bft), lhsT2=lhsT2.astype(bft),
                lhsT3=lhsT3.astype(bft),
                b1blk=b1blk, b2blk=b2blk, b3blk=b3blk)


def kernel(**inputs):
    global B3C
    inputs = {k: np.asarray(v, np.float32) for k, v in inputs.items()}
    b3c = np.concatenate([inputs['mu_b3'], inputs['ld_b3'], inputs['lo_b3']])
    B3C = [float(x) for x in b3c]
    packed = _pack_host(inputs)
    y_fm_all = np.ascontiguousarray(inputs['y'].T)          # [2, N]
    zs_all = inputs['zs'].reshape(N_TOT, 32)

    in_maps = []
    for c in range(NCORES):
        a, b = c * NS, (c + 1) * NS
        m = dict(packed)
        m['y_fm'] = np.ascontiguousarray(y_fm_all[:, a:b]).astype(ml_dtypes.bfloat16)
        m['zs'] = zs_all[a:b]
        in_maps.append(m)

    nc = _build_nc()
    res = run_bass_kernel_spmd(nc, in_maps, core_ids=list(range(NCORES)))
    ssq = sent = 0.0
    for r in res.results:
        ssq += float(r['out'][0, 0])
        sent += float(r['out'][0, 1])

    ln2pi = float(np.log(2.0 * np.pi))
    prior_c = -float(np.log(0.25) + 3 * np.log(0.5)) - 2.0 * ln2pi
    lik_c = 2.0 * (-float(np.log(0.01)) - 0.5 * ln2pi)
    ent_c = 0.5 * 4 * (1.0 + ln2pi)
    C = prior_c + lik_c + ent_c
    val = -ssq / (N_TOT * P) + sent / N_TOT + C
    return np.float32(val)


# revision 39
# speedup vs baseline: 1.0254x; 1.0254x over previous
"""Amortized-VI loss kernel for 8 TRN2 NeuronCores (self-contained).

Pure data-parallel over N=524288 samples (65536/core). The scalar loss
decomposes into 3 global sums computed on-chip; host combines 8 per-core
partials:
  S_sq  = sum_{n,p} [ sum_i c_i*xi_i^2 + 5000*((y0-px)^2 + (y1-py)^2) ]
  S_ent = sum_n sum_i ln(softplus(ld_i))
  final = -S_sq/(N*P) + S_ent/N + C

Per core, pipelined in 4 "pairs" (each = 4 MLP super-chunks + 2 particle
chunks) so the per-sample MLP (PE+ACT) overlaps the per-(n,p) DVE work:
  Phase A: 3 tiny MLPs via 2-group block-diag matmuls; relu+bias on ACT;
    outputs bounced through DRAM (fm_dram) with columns ordered
    (g_local, s') so phase-B chunks depend only on their own super-chunks.
  Phase B: einsum xi = mu + L z on DVE with step-0 broadcast APs,
    int-round range reduction + Sin/Abs on ACT, fused square+accumulate
    via scalar_tensor_tensor accum_out.
"""
import sys
import numpy as np
import ml_dtypes

sys.path.insert(0, '/opt/trn_rl_repo')

import concourse.bass as bass
import concourse.tile as tile
from concourse import bacc, mybir
from concourse.bass_utils import run_bass_kernel_spmd
from contextlib import ExitStack

F32 = mybir.dt.float32
F32R = mybir.dt.float32r
BF = mybir.dt.bfloat16
I32 = mybir.dt.int32
AF = mybir.ActivationFunctionType
OP = mybir.AluOpType

NCORES = 8
N_TOT = 524288
NS = N_TOT // NCORES          # 65536 samples/core
P = 8                         # particles
G = 128                       # partition groups (g = n // 512)
S = NS // G                   # 512 samples per group
NCH = 8                       # phase-B chunks (s-slices of 64)
CH = S // NCH                 # 64
PPAIRS = NCH // 2             # 4 pipelined pairs
SC = 16                       # phase-A super-chunks (4 per pair)
SPS = S // SC                 # 32 s-columns per super-chunk
SCW = 64 * SPS                # 2048 matmul cols (64 g_local x 32 s')
TPS = SCW // 512              # 4 matmul tiles per super-chunk

PI = float(np.pi)
TWO_PI = 2.0 * PI
C_PRIOR = [8.0, 2.0, 2.0, 2.0]   # 0.5/sigma_i^2
C_LIK = 5000.0                   # 0.5/noise^2

# L channel order: L00=d0; L10=lo0 L11=d1; L20=lo1 L21=lo2 L22=d2;
# L30=lo3 L31=lo4 L32=lo5 L33=d3
L_TERMS = {0: [(0, 'd0')],
           1: [(0, 'lo0'), (1, 'd1')],
           2: [(0, 'lo1'), (1, 'lo2'), (2, 'd2')],
           3: [(0, 'lo3'), (1, 'lo4'), (2, 'lo5'), (3, 'd3')]}

CHANS = (['mu0', 'mu1', 'mu2', 'mu3'] + [f'ld{i}' for i in range(4)]
         + [f'lo{i}' for i in range(6)])

_CACHE = {}
B3C = [0.0] * 14   # per-channel L3 bias, set by kernel() before build


def _build_nc():
    key = ('nc',) + tuple(B3C)
    if key in _CACHE:
        return _CACHE[key]
    nc = bacc.Bacc("TRN2", target_bir_lowering=False, debug=False)
    dp = lambda n, s: nc.declare_dram_parameter(n, s, F32, isOutput=False)
    dpb = lambda n, s: nc.declare_dram_parameter(n, s, BF, isOutput=False)
    y_fm = dpb("y_fm", [2, NS])
    zs = dp("zs", [NS, 32])
    lhsT1 = dpb("lhsT1", [4, 120])
    lhsT2 = dpb("lhsT2", [120, 60])
    lhsT3 = dpb("lhsT3", [60, 28])
    b1blk = dp("b1blk", [120, 1])
    b2blk = dp("b2blk", [60, 1])
    b3blk = dp("b3blk", [28, 1])
    out_d = nc.declare_dram_parameter("out", [1, 2], F32, isOutput=True)
    fm_dram = nc.dram_tensor("fm_dram", [SC, 28, SCW], BF)

    with tile.TileContext(nc) as tc:
        with ExitStack() as ctx:
            wpool = ctx.enter_context(tc.tile_pool(name="wpool", bufs=1))
            apool = ctx.enter_context(tc.tile_pool(name="apool", bufs=2))
            ps1p = ctx.enter_context(tc.tile_pool(name="ps1", bufs=2, space="PSUM"))
            ps2p = ctx.enter_context(tc.tile_pool(name="ps2", bufs=1, space="PSUM"))
            ps3p = ctx.enter_context(tc.tile_pool(name="ps3", bufs=1, space="PSUM"))
            ctp = ctx.enter_context(tc.tile_pool(name="ctp", bufs=3))
            zpool = ctx.enter_context(tc.tile_pool(name="zpool", bufs=3))
            bpool = ctx.enter_context(tc.tile_pool(name="bpool", bufs=1))
            b2pool = ctx.enter_context(tc.tile_pool(name="b2pool", bufs=2))

            # ---- weights + constants ----
            w1 = wpool.tile([4, 120], BF); nc.sync.dma_start(w1[:], lhsT1[:])
            w2 = wpool.tile([120, 60], BF); nc.sync.dma_start(w2[:], lhsT2[:])
            w3 = wpool.tile([60, 28], BF); nc.sync.dma_start(w3[:], lhsT3[:])
            b1 = wpool.tile([120, 1], F32); nc.sync.dma_start(b1[:], b1blk[:])
            b2 = wpool.tile([60, 1], F32); nc.sync.dma_start(b2[:], b2blk[:])
            kone = wpool.tile([128, 1], F32); nc.vector.memset(kone[:], 1.0)
            khpi = wpool.tile([128, 1], F32); nc.vector.memset(khpi[:], PI / 2)
            accsq = wpool.tile([G, 48], F32); nc.vector.memset(accsq[:], 0.0)
            accent = wpool.tile([G, 16], F32); nc.vector.memset(accent[:], 0.0)

            # y_fm views
            yg = [y_fm[k:k + 1, :].rearrange("a (g s) -> (a g) s", g=G)
                  for k in range(2)]          # [128, 512] DRAM view
            ysc = [y_fm[k:k + 1, :].rearrange(
                "a (grp gl s) -> (a grp) gl s", grp=2, gl=64) for k in range(2)]

            for pp in range(PPAIRS):
                # ===== Phase A: 4 super-chunks =====
                for j in range(TPS):
                    sc = pp * 4 + j
                    yT = apool.tile([4, SCW], BF, tag="yT")
                    for k in range(2):
                        nc.sync.dma_start(
                            yT[2 * k:2 * k + 2, :].rearrange(
                                "r (gl s) -> r gl s", gl=64),
                            ysc[k][:, :, sc * SPS:(sc + 1) * SPS])
                    h1 = apool.tile([120, SCW], BF, tag="h1")
                    h2 = apool.tile([60, SCW], BF, tag="h2")
                    fm = apool.tile([28, SCW], BF, tag="fm")
                    for t2 in range(TPS // 2):
                        cs = slice(t2 * 1024, (t2 + 1) * 1024)
                        p1 = ps1p.tile([120, 1024], F32, tag="p1")
                        for u in range(2):
                            nc.tensor.matmul(p1[:, u * 512:(u + 1) * 512], w1[:],
                                             yT[:, t2 * 1024 + u * 512:
                                                t2 * 1024 + (u + 1) * 512],
                                             start=True, stop=True)
                        nc.scalar.activation(h1[:, cs], p1[:], AF.Relu, bias=b1[:])
                    for t2 in range(TPS // 2):
                        cs = slice(t2 * 1024, (t2 + 1) * 1024)
                        p2 = ps2p.tile([60, 1024], F32, tag="p2")
                        for u in range(2):
                            nc.tensor.matmul(p2[:, u * 512:(u + 1) * 512], w2[:],
                                             h1[:, t2 * 1024 + u * 512:
                                                t2 * 1024 + (u + 1) * 512],
                                             start=True, stop=True)
                        nc.scalar.activation(h2[:, cs], p2[:], AF.Relu, bias=b2[:])
                    for t2 in range(TPS // 2):
                        cs = slice(t2 * 1024, (t2 + 1) * 1024)
                        p3 = ps3p.tile([28, 1024], F32, tag="p3")
                        for u in range(2):
                            nc.tensor.matmul(p3[:, u * 512:(u + 1) * 512], w3[:],
                                             h2[:, t2 * 1024 + u * 512:
                                                t2 * 1024 + (u + 1) * 512],
                                             start=True, stop=True)
                        nc.scalar.activation(fm[:, cs], p3[:], AF.Copy)
                    nc.sync.dma_start(fm_dram[sc], fm[:])

                # ===== gather pair channel tiles (coalesced) =====
                call = ctp.tile([G, 14, 2 * CH], BF, tag="call")
                for j in range(4):
                    sc = 4 * pp + j
                    for grp in range(2):
                        src = fm_dram[sc, 14 * grp:14 * grp + 14, :]\
                            .rearrange("r (gl sp) -> gl r sp", sp=SPS)
                        dst = call[grp * 64:(grp + 1) * 64, :,
                                   j * SPS:(j + 1) * SPS]
                        eng = nc.sync if (j % 2 == 0) else nc.gpsimd
                        eng.dma_start(dst, src)
                cpt = {chan: call[:, ci] for ci, chan in enumerate(CHANS)}
                y0p = ctp.tile([G, 2 * CH], BF, tag="y0p")
                y1p = ctp.tile([G, 2 * CH], BF, tag="y1p")
                nc.gpsimd.dma_start(y0p[:], yg[0][:, pp * 2 * CH:(pp + 1) * 2 * CH])
                nc.gpsimd.dma_start(y1p[:], yg[1][:, pp * 2 * CH:(pp + 1) * 2 * CH])

                # pre-bias mu and lo channels (b3 immediates), bf16
                mub = {}
                lob = {}
                for i in range(4):
                    mt = ctp.tile([G, 2 * CH], BF, tag=f"mub{i}", name=f"mub{i}")
                    nc.vector.tensor_scalar_add(mt[:], cpt[f'mu{i}'], float(B3C[i]))
                    mub[i] = mt
                for i in range(6):
                    lt_ = ctp.tile([G, 2 * CH], BF, tag=f"lob{i}", name=f"lob{i}")
                    nc.vector.tensor_scalar_add(lt_[:], cpt[f'lo{i}'],
                                                float(B3C[8 + i]))
                    lob[i] = lt_

                # ===== softplus(ld+b) -> diag + entropy (per pair) =====
                dgp = {}
                ldb = {}
                ab2 = {}
                for i in range(4):
                    lt = ctp.tile([G, 2 * CH], F32, tag=f"ldb{i}", name=f"ldb{i}")
                    at_ = ctp.tile([G, 2 * CH], F32, tag=f"spab{i}", name=f"spab{i}")
                    nc.vector.tensor_scalar_add(lt[:], cpt[f'ld{i}'],
                                                float(B3C[4 + i]))
                    nc.vector.scalar_tensor_tensor(at_[:], lt[:], -1.0, lt[:],
                                                   OP.mult, OP.max)
                    ldb[i] = lt; ab2[i] = at_
                exn = {}
                lnd = {}
                if True:
                    for i in range(4):
                        et = ctp.tile([G, 2 * CH], F32, tag=f"spex{i}",
                                      name=f"spex{i}")
                        nc.scalar.activation(et[:], ab2[i][:], AF.Exp, scale=-1.0)
                        exn[i] = et
                    ln1 = {}
                    for i in range(4):
                        lt1 = ctp.tile([G, 2 * CH], F32, tag=f"spln{i}",
                                       name=f"spln{i}")
                        nc.scalar.activation(lt1[:], exn[i][:], AF.Ln,
                                             bias=kone[:])
                        ln1[i] = lt1
                    for i in range(4):
                        dgt = ctp.tile([G, 2 * CH], BF, tag=f"dg{i}",
                                       name=f"dg{i}")
                        rel = ctp.tile([G, 2 * CH], F32, tag=f"sp_re{i}",
                                       name=f"sp_re{i}")
                        nc.vector.tensor_scalar_max(rel[:], ldb[i][:], 0.0)
                        nc.vector.tensor_add(dgt[:], rel[:], ln1[i][:])
                        dgp[i] = dgt
                    for i in range(4):
                        lt2 = ctp.tile([G, 2 * CH], F32, tag=f"splnd{i}",
                                       name=f"splnd{i}")
                        nc.scalar.activation(lt2[:], dgp[i][:], AF.Ln)
                        lnd[i] = lt2
                for i in range(4):
                    nc.vector.tensor_reduce(accent[:, pp * 4 + i: pp * 4 + i + 1],
                                            lnd[i][:], mybir.AxisListType.X, OP.add)

                # ===== Phase B: 2 chunks =====
                for cl in range(2):
                    c = pp * 2 + cl
                    col = slice(cl * CH, (cl + 1) * CH)
                    zt = zpool.tile([G, CH, P, 4], BF, tag="zt")
                    nc.gpsimd.dma_start(
                        zt[:], zs[:].rearrange("(g c s) j -> g c (s j)",
                                               g=G, c=NCH)[:, c]
                        .rearrange("g (s p j) -> g s p j", p=P, j=4))

                    def bc(t):   # [G, 2CH] pair tile/AP -> [G, 8, CH] bcast
                        return t[:, col].unsqueeze(1).broadcast_to([G, P, CH])

                    # deinterleave z on GPSIMD: dense bf16 z_j tiles
                    zd = zpool.tile([G, 4, P, CH], BF, tag="zd")
                    for j in range(4):
                        nc.gpsimd.tensor_copy(
                            zd[:, j], zt[:, :, :, j].transpose([0, 2, 1]))

                    def zj(j):   # [G, 8, CH] dense z_j
                        return zd[:, j]

                    def lch(src):
                        t = dgp[int(src[1])] if src[0] == 'd' else lob[int(src[2])]
                        return t[:, col].unsqueeze(1).broadcast_to([G, P, CH])

                    atile = b2pool.tile([G, 3, P, CH], F32, tag="atile")
                    xi0 = b2pool.tile([G, P, CH], BF, tag="xi0")
                    xi2 = b2pool.tile([G, P, CH], BF, tag="xi2")
                    xi3 = b2pool.tile([G, P, CH], BF, tag="xi3")
                    qt = b2pool.tile([G, P, CH], BF, tag="qt")
                    # einsum xi_i = mu_i + sum_j L_ij z_j
                    # i=1 goes straight into atile[:,0]
                    for i, dstap in ((0, xi0[:]), (1, atile[:, 0]),
                                     (2, xi2[:]), (3, xi3[:])):
                        acc = None
                        for ti, (j, src) in enumerate(L_TERMS[i]):
                            tgt = dstap if acc is None else qt[:]
                            eng = nc.gpsimd if (i == 3 and ti < 2) else nc.vector
                            eng.tensor_tensor(tgt, zj(j), lch(src), OP.mult)
                            if acc is not None:
                                nc.vector.tensor_tensor(dstap, dstap, qt[:], OP.add)
                            acc = dstap
                        nc.vector.tensor_tensor(dstap, dstap, bc(mub[i]), OP.add)
                    nc.vector.tensor_tensor(atile[:, 1], atile[:, 0], xi2[:], OP.add)
                    nc.vector.tensor_tensor(atile[:, 2], atile[:, 1], xi3[:], OP.add)
                    # range reduction: k = round(a/2pi), q = a - 2pi*k
                    ki = b2pool.tile([G, 3, P, CH], I32, tag="ki")
                    qt3 = b2pool.tile([G, 3, P, CH], F32, tag="qt3")
                    nc.vector.tensor_scalar(ki[:], atile[:], float(1.0 / TWO_PI),
                                            None, OP.mult)
                    nc.vector.scalar_tensor_tensor(qt3[:], ki[:], -TWO_PI,
                                                   atile[:], OP.mult, OP.add)
                    st = b2pool.tile([G, 3, P, CH], BF, tag="st")
                    ab = b2pool.tile([G, 3, P, CH], F32, tag="ab")
                    co = b2pool.tile([G, 3, P, CH], BF, tag="co")
                    nc.scalar.activation(st[:], qt3[:], AF.Sin)
                    nc.scalar.activation(ab[:], qt3[:], AF.Abs)
                    nc.scalar.activation(co[:], ab[:], AF.Sin, bias=khpi[:],
                                         scale=-1.0)

                    sa = lambda k: st[:, k]
                    ca = lambda k: co[:, k]
                    uu = b2pool.tile([G, P, CH], BF, tag="uu")
                    t1 = b2pool.tile([G, P, CH], BF, tag="t1")
                    ex = b2pool.tile([G, P, CH], BF, tag="ex")
                    sq = b2pool.tile([G, P, CH], BF, tag="sq")
                    nc.vector.tensor_tensor(uu[:], ca(0), ca(1), OP.add)
                    nc.vector.scalar_tensor_tensor(t1[:], ca(2), 2.0, uu[:],
                                                   OP.mult, OP.add)
                    nc.vector.scalar_tensor_tensor(ex[:], t1[:], -0.5, bc(y0p),
                                                   OP.mult, OP.add)
                    nc.vector.scalar_tensor_tensor(
                        sq[:], ex[:], C_LIK, ex[:], OP.mult, OP.mult,
                        accum_out=accsq[:, c * 6: c * 6 + 1])
                    nc.vector.tensor_tensor(uu[:], sa(0), sa(1), OP.add)
                    nc.vector.scalar_tensor_tensor(t1[:], sa(2), 2.0, uu[:],
                                                   OP.mult, OP.add)
                    nc.vector.scalar_tensor_tensor(ex[:], xi0[:], -1.0, bc(y1p),
                                                   OP.mult, OP.add)
                    nc.vector.scalar_tensor_tensor(t1[:], t1[:], -0.5, ex[:],
                                                   OP.mult, OP.add)
                    nc.vector.scalar_tensor_tensor(
                        sq[:], t1[:], C_LIK, t1[:], OP.mult, OP.mult,
                        accum_out=accsq[:, c * 6 + 1: c * 6 + 2])
                    for i, xt in ((0, xi0[:]), (1, atile[:, 0]),
                                  (2, xi2[:]), (3, xi3[:])):
                        nc.vector.scalar_tensor_tensor(
                            sq[:], xt, C_PRIOR[i], xt, OP.mult, OP.mult,
                            accum_out=accsq[:, c * 6 + 2 + i: c * 6 + 3 + i])

            # ===== final reduction =====
            red = wpool.tile([G, 2], F32)
            nc.vector.tensor_reduce(red[:, 0:1], accsq[:], mybir.AxisListType.X,
                                    OP.add)
            nc.vector.tensor_reduce(red[:, 1:2], accent[:], mybir.AxisListType.X,
                                    OP.add)
            pf = ps3p.tile([1, 2], F32, tag="p3")
            nc.tensor.matmul(pf[:], kone[:], red[:], start=True, stop=True)
            ob = wpool.tile([1, 2], F32)
            nc.vector.tensor_copy(ob[:], pf[:])
            nc.sync.dma_start(out_d[:], ob[:])

    nc.compile()
    _CACHE[key] = nc
    return nc


def _pack_host(inp):
    cat = np.concatenate
    W1c = cat([inp['mu_W1'], inp['ld_W1'], inp['lo_W1']], axis=1)      # [2,60]
    b1c = cat([inp['mu_b1'], inp['ld_b1'], inp['lo_b1']])              # [60]
    lhsT1 = np.zeros((4, 120), np.float32)
    for k in range(2):
        for g in range(2):
            lhsT1[2 * k + g, 60 * g:60 * (g + 1)] = W1c[k]
    b1blk = np.tile(b1c, 2).reshape(120, 1).astype(np.float32)

    def blkdiag(ws):
        r = sum(w.shape[0] for w in ws); c = sum(w.shape[1] for w in ws)
        out = np.zeros((r, c), np.float32)
        ro = co = 0
        for w in ws:
            out[ro:ro + w.shape[0], co:co + w.shape[1]] = w
            ro += w.shape[0]; co += w.shape[1]
        return out

    W2b = blkdiag([inp['mu_W2'], inp['ld_W2'], inp['lo_W2']])          # [60,30]
    b2c = cat([inp['mu_b2'], inp['ld_b2'], inp['lo_b2']])              # [30]
    lhsT2 = np.zeros((120, 60), np.float32)
    lhsT2[0:60, 0:30] = W2b; lhsT2[60:120, 30:60] = W2b
    b2blk = np.tile(b2c, 2).reshape(60, 1).astype(np.float32)

    W3b = blkdiag([inp['mu_W3'], inp['ld_W3'], inp['lo_W3']])          # [30,14]
    lhsT3 = np.zeros((60, 28), np.float32)
    lhsT3[0:30, 0:14] = W3b; lhsT3[30:60, 14:28] = W3b
    b3c = cat([inp['mu_b3'], inp['ld_b3'], inp['lo_b3']])              # [14]
    b3blk = np.tile(b3c, 2).reshape(28, 1).astype(np.float32)
    bft = ml_dtypes.bfloat16
    return dict(lhsT1=lhsT1.astype(bft), lhsT2=lhsT2.astype(bft),
                lhsT3=lhsT3.astype(bft),
                b1blk=b1blk, b2blk=b2blk, b3blk=b3blk)


def kernel(**inputs):
    global B3C
    inputs = {k: np.asarray(v, np.float32) for k, v in inputs.items()}
    b3c = np.concatenate([inputs['mu_b3'], inputs['ld_b3'], inputs['lo_b3']])
    B3C = [float(x) for x in b3c]
    packed = _pack_host(inputs)
    y_fm_all = np.ascontiguousarray(inputs['y'].T)          # [2, N]
    zs_all = inputs['zs'].reshape(N_TOT, 32)

    in_maps = []
    for c in range(NCORES):
        a, b = c * NS, (c + 1) * NS
        m = dict(packed)
        m['y_fm'] = np.ascontiguousarray(y_fm_all[:, a:b]).astype(ml_dtypes.bfloat16)
        m['zs'] = zs_all[a:b]
        in_maps.append(m)

    nc = _build_nc()
    res = run_bass_kernel_spmd(nc, in_maps, core_ids=list(range(NCORES)))
    ssq = sent = 0.0
    for r in res.results:
        ssq += float(r['out'][0, 0])
        sent += float(r['out'][0, 1])

    ln2pi = float(np.log(2.0 * np.pi))
    prior_c = -float(np.log(0.25) + 3 * np.log(0.5)) - 2.0 * ln2pi
    lik_c = 2.0 * (-float(np.log(0.01)) - 0.5 * ln2pi)
    ent_c = 0.5 * 4 * (1.0 + ln2pi)
    C = prior_c + lik_c + ent_c
    val = -ssq / (N_TOT * P) + sent / N_TOT + C
    return np.float32(val)
